# revision 24
# baseline (speedup 1.0000x reference)
"""GQA attention (B=2, T=2048, HID=2048, 32 q-heads / 8 kv-heads, d=64)
distributed over 8 TRN2 NeuronCores.

Sharding: tensor-parallel over heads. Core c owns q-heads [4c, 4c+4) and
kv-head c (column shards of Wq/Wk/Wv), plus the matching column shard of Wo
used to compute out^T rows. x is replicated (host pre-transposes to [hid, tok]
and casts to bf16). After each head-pair the core AllGathers its y^T
[128, 2048] block; the out projection consumes k-chunks in arrival order and
emits outT[256c:256c+256, :] in bf16. The host concatenates and transposes.

Phase 2 pipeline: per key chunk j, scores land in [128, 1024] PSUM chunks
(double buffered), exp runs on ACT into a [128, 2048] pT tile, the causal
triangle is zeroed afterwards with affine_select on gpsimd (so no mask adds
and no mask input), and PV accumulates into two [65, 1024] y_acc halves that
free as soon as each half is normalized. The softmax denominator comes from a
ones-column appended to V; its reciprocal is computed at [128, 8] shape via a
DMA round-trip (reciprocal on a [1, 1024] row is ~15 us on DVE, the
transposed form is free).
"""

import os
import sys

import numpy as np

for _p in ("/opt/trn_rl_repo", "/root/.axon_site/_ro/trn_rl_repo"):
    if os.path.isdir(_p) and _p not in sys.path:
        sys.path.append(_p)

import ml_dtypes  # noqa: E402
from contextlib import ExitStack  # noqa: E402

import concourse.bass as bass  # noqa: E402
import concourse.tile as tile  # noqa: E402
from concourse import bacc, mybir  # noqa: E402
from concourse.bass_utils import run_bass_kernel_spmd  # noqa: E402

BF16 = mybir.dt.bfloat16
F32 = mybir.dt.float32
NPBF16 = ml_dtypes.bfloat16

B, T, HID = 2, 2048, 2048
NT = B * T
HEADS, KV_HEADS, DH = 32, 8, 64
NCORES = 8
QH = HEADS // NCORES          # q-heads per core
DQ = QH * DH                  # 256
KC = HID // 128               # 16 hidden-dim chunks
JC = T // 128                 # 16 key chunks of 128 per batch
HC = T // 1024                # 2 q-column halves of 1024 per batch
EXP = mybir.ActivationFunctionType.Exp


def _build(mode: str, debug: bool = False, compile: bool = True) -> bacc.Bacc:
    """mode: 'causal' (128-granular trim + affine triangle),
    'zeros' (no mask work), 'general' (additive mask from DRAM)."""
    causal = mode == "causal"
    nc = bacc.Bacc(
        "TRN2", target_bir_lowering=False, debug=debug, num_devices=NCORES
    )
    xT = nc.dram_tensor("xT", [B, KC, 128, T], BF16, kind="ExternalInput")
    wq = nc.dram_tensor("wq", [KC, 128, DQ], BF16, kind="ExternalInput")
    wkv = nc.dram_tensor("wkv", [KC, 128, 128], BF16, kind="ExternalInput")
    wo = nc.dram_tensor("wo", [KC, 128, DQ], BF16, kind="ExternalInput")
    ident = nc.dram_tensor("ident", [128, 128], BF16, kind="ExternalInput")
    if mode == "general":
        maskT = nc.dram_tensor("maskT", [JC, 128, T], BF16, kind="ExternalInput")
    outT = nc.dram_tensor("outT", [2, 128, NT], BF16, kind="ExternalOutput")

    def qlo(j):  # first valid q column for key chunk j
        return 128 * j if causal else 0

    with tile.TileContext(nc) as tc, ExitStack() as top:
        wpool = top.enter_context(tc.tile_pool(name="weights", bufs=1))
        wq_sb = wpool.tile([128, KC, DQ], BF16)
        wkv_sb = wpool.tile([128, KC, 128], BF16)
        nc.gpsimd.dma_start(wq_sb[:], wq[:, :, :].rearrange("k p d -> p k d"))
        nc.gpsimd.dma_start(wkv_sb[:], wkv[:, :, :].rearrange("k p d -> p k d"))

        qkv_pool = top.enter_context(tc.tile_pool(name="qkv", bufs=1))
        qT = [qkv_pool.tile([64, NT], BF16, name=f"qT{h}") for h in range(QH)]
        kT = qkv_pool.tile([64, NT], BF16, name="kT")
        vT = qkv_pool.tile([64, NT], BF16, name="vT")
        vones = qkv_pool.tile([128, B, JC, DH + 1], BF16, name="vones")
        yT_sb = [qkv_pool.tile([128, NT], BF16, name=f"yTsb{i}") for i in range(2)]
        ident_sb = wpool.tile([128, 128], BF16, name="ident_sb")
        ones_sb = wpool.tile([1, 64], BF16, name="ones_sb")
        nc.gpsimd.dma_start(ident_sb[:], ident[:])
        nc.vector.memset(ones_sb[:], 1.0)
        nc.vector.memset(vones[:, :, :, DH : DH + 1], 1.0)
        # 0/1 keep-mask for the causal in-block triangle: 1 iff qq >= kk
        tri01 = wpool.tile([128, 128], BF16, name="tri01")
        nc.gpsimd.memset(tri01[:], 1.0)
        nc.gpsimd.affine_select(
            out=tri01[:],
            in_=tri01[:],
            compare_op=mybir.AluOpType.is_ge,
            fill=0.0,
            base=0,
            pattern=[[1, 128]],
            channel_multiplier=-1,
        )

        # ---------------- phase 1: QKV projections (transposed layout) ------
        with tc.tile_pool(name="xcol", bufs=2) as xpool, tc.tile_pool(
            name="qkvps", bufs=3, space="PSUM"
        ) as qkvps:
            for n in range(B * 2):
                b, nn = divmod(n, 2)
                gcol = slice(b * T + nn * 1024, b * T + nn * 1024 + 1024)
                xc = xpool.tile([128, KC, 1024], BF16, name="xc")
                nc.gpsimd.dma_start(
                    xc[:],
                    xT[b, :, :, nn * 1024 : (nn + 1) * 1024].rearrange(
                        "k p t -> p k t"
                    ),
                )
                for m in range(2):  # q-head pairs (2m, 2m+1)
                    ps = qkvps.tile([128, 1024], F32, name="ps")
                    for u in (0, 512):  # matmul out must stay in one PSUM bank
                        for k in range(KC):
                            nc.tensor.matmul(
                                ps[:, u : u + 512],
                                wq_sb[:, k, m * 128 : (m + 1) * 128],
                                xc[:, k, u : u + 512],
                                start=(k == 0),
                                stop=(k == KC - 1),
                            )
                    nc.vector.tensor_copy(qT[2 * m][0:64, gcol], ps[0:64, :])
                    nc.vector.tensor_copy(qT[2 * m + 1][0:64, gcol], ps[64:128, :])
                ps = qkvps.tile([128, 1024], F32, name="ps")
                for u in (0, 512):
                    for k in range(KC):
                        nc.tensor.matmul(
                            ps[:, u : u + 512],
                            wkv_sb[:, k, :],
                            xc[:, k, u : u + 512],
                            start=(k == 0),
                            stop=(k == KC - 1),
                        )
                nc.vector.tensor_copy(kT[0:64, gcol], ps[0:64, :])
                nc.vector.tensor_copy(vT[0:64, gcol], ps[64:128, :])

        # ---------------- phase 1.5: V to natural layout (PE transpose) -----
        with tc.tile_pool(name="tps", bufs=3, space="PSUM") as tpool:
            for b in range(B):
                for j in range(JC):
                    tp = tpool.tile([128, DH], BF16, name="tp")
                    nc.tensor.transpose(
                        tp[:],
                        vT[0:64, b * T + j * 128 : b * T + (j + 1) * 128],
                        ident_sb[0:64, 0:64],
                    )
                    nc.vector.tensor_copy(vones[:, b, j, 0:DH], tp[:])

        # DRAM bounce buffers for the per-head-pair AllGathers.
        dpool = top.enter_context(tc.tile_pool(name="dram", bufs=1, space="DRAM"))
        yT_in = [
            [dpool.tile([128, T], BF16, name=f"yTin{b}_{i}") for i in range(2)]
            for b in range(B)
        ]
        yT_all = [
            [
                dpool.tile(
                    [NCORES, 128, T], BF16, addr_space="Shared", name=f"yTall{b}_{i}"
                )
                for i in range(2)
            ]
            for b in range(B)
        ]

        # Out-projection SBUF pools open before attention so wo and the first
        # y^T k-chunks prefetch while attention still runs.
        ylp = top.enter_context(tc.tile_pool(name="ysl", bufs=4))
        wop = top.enter_context(tc.tile_pool(name="wopool", bufs=1))
        wo_sb = wop.tile([128, KC, DQ], BF16, name="wo_sb")
        nc.gpsimd.dma_start(wo_sb[:], wo[:, :, :].rearrange("k p d -> p k d"))
        prefetched = {}

        def load_ysl(b, kc):
            t = ylp.tile([128, T], BF16, name="ysl")
            nc.gpsimd.dma_start(t[:], yT_all[b][kc % 2][kc // 2, :, :])
            return t

        # ---------------- phase 2: attention ------------------------------
        with tc.tile_pool(name="spool", bufs=2, space="PSUM") as spool, tc.tile_pool(
            name="ypool", bufs=1, space="PSUM"
        ) as ypsum, tc.tile_pool(name="ppool", bufs=3) as ppool, tc.tile_pool(
            name="npool", bufs=2
        ) as npool, (
            tc.tile_pool(name="mpool", bufs=1) if mode == "general" else ExitStack()
        ) as mpool:
            if mode == "general":
                mask_sb = mpool.tile([128, JC, T], BF16, name="mask_sb")
                nc.gpsimd.dma_start(
                    mask_sb[:], maskT[:, :, :].rearrange("j p w -> p j w")
                )

            def normalize_start(b, h, half, y_acc):
                """Copy den row out, kick the reciprocal round-trip. Returns
                state for normalize_finish. For half 1 also copy y to SBUF so
                the y_acc slot frees before the next head's first PV."""
                den_sb = npool.tile([1, 1024], F32, name="den_sb", tag="den")
                nc.vector.tensor_copy(den_sb[:], y_acc[DH : DH + 1, :])
                # copy y out of PSUM: frees the y_acc slot early AND the
                # final mul may only read one PSUM operand (rb_ps)
                yu = npool.tile([64, 1024], BF16, name="yu", tag="yu")
                nc.vector.tensor_copy(yu[:], y_acc[0:DH, :])
                den_t = npool.tile([128, 8], F32, name="den_t", tag="den_t")
                nc.gpsimd.dma_start(den_t[0:128, 0:8], den_sb[0:1, 0:1024])
                r_t = npool.tile([128, 8], F32, name="r_t", tag="r_t")
                nc.vector.reciprocal_approx_fast(r_t[:], den_t[:])
                r_row = npool.tile([1, 1024], BF16, name="r_row", tag="r_row")
                nc.gpsimd.dma_start(r_row[0:1, 0:1024], r_t[0:128, 0:8])
                return (b, h, half, y_acc, yu, r_row)

            def normalize_finish(state):
                b, h, half, y_acc, yu, r_row = state
                rb_ps = spool.tile([128, 1024], F32, name="rb_ps", tag="S")
                for u in (0, 512):
                    nc.tensor.matmul(
                        rb_ps[0:64, u : u + 512],
                        ones_sb[:],
                        r_row[0:1, u : u + 512],
                        start=True,
                        stop=True,
                    )
                dst = yT_sb[h // 2][
                    64 * (h % 2) : 64 * (h % 2) + 64,
                    b * T + half * 1024 : b * T + half * 1024 + 1024,
                ]
                nc.vector.tensor_mul(dst, yu[:], rb_ps[0:64, :])

            def allgather(b, i):
                nc.gpsimd.dma_start(
                    yT_in[b][i][:], yT_sb[i][:, b * T : (b + 1) * T]
                )
                nc.gpsimd.collective_compute(
                    "AllGather",
                    mybir.AluOpType.bypass,
                    replica_groups=[list(range(NCORES))],
                    ins=[yT_in[b][i].opt()],
                    outs=[yT_all[b][i].opt()],
                )
                # prefetch after AG2 fires: by then AG1 has completed, so the
                # in-order gpsimd queue won't block later den DMAs on it
                if (b, i) == (0, 1):
                    for kc in (0, 2, 4):
                        prefetched[(0, kc)] = load_ysl(0, kc)

            pending = []   # deferred normalize_finish / allgather thunks
            deferred = []
            for b in range(B):
                for h in range(QH):
                    y_acc = [
                        ypsum.tile([DH + 1, 1024], F32, name=f"yacc{c}", tag=f"yacc{c}")
                        for c in range(2)
                    ]
                    for j in range(JC):
                        q0 = qlo(j)
                        pT = ppool.tile([128, T], BF16, name="pT")
                        lk = kT[0:64, b * T + j * 128 : b * T + (j + 1) * 128]
                        for half in range(2):
                            lo, hi = half * 1024, half * 1024 + 1024
                            qs = max(lo, q0)
                            if qs >= hi:
                                continue
                            w = hi - qs
                            S = spool.tile([128, 1024], F32, name="S")
                            for p0 in range(0, w, 512):
                                pw = min(512, w - p0)
                                nc.tensor.matmul(
                                    S[:, p0 : p0 + pw],
                                    lk,
                                    qT[h][
                                        0:64,
                                        b * T + qs + p0 : b * T + qs + p0 + pw,
                                    ],
                                    start=True,
                                    stop=True,
                                )
                            if mode == "general":
                                nc.vector.tensor_add(
                                    S[:, 0:w], S[:, 0:w], mask_sb[:, j, qs:hi]
                                )
                            nc.scalar.activation(pT[:, qs:hi], S[:, 0:w], EXP)
                            if causal and qs == q0:
                                # zero the in-block triangle: keep col qq of
                                # partition kk iff qq >= kk
                                nc.vector.tensor_mul(
                                    pT[:, q0 : q0 + 128],
                                    pT[:, q0 : q0 + 128],
                                    tri01[:],
                                )
                        for half in range(2):
                            lo, hi = half * 1024, half * 1024 + 1024
                            qs = max(lo, q0)
                            if qs >= hi:
                                continue
                            a = qs - lo
                            bounds = [a, 512, 1024] if a < 512 else [a, 1024]
                            pieces = list(zip(bounds, bounds[1:]))
                            if qs == q0:
                                # diag piece last: its pT slice waits on the
                                # triangle zeroing; the other piece doesn't
                                pieces = pieces[::-1]
                            for pa, pb in pieces:
                                nc.tensor.matmul(
                                    y_acc[half][:, pa:pb],
                                    vones[:, b, j, :],
                                    pT[:, lo + pa : lo + pb],
                                    start=(j == 0),
                                    stop=(j == JC - 1 or (half == 0 and j == 7)),
                                    skip_group_check=True,
                                )
                        # half 0 complete once j==7 has accumulated (causal);
                        # start its normalization so the reciprocal round-trip
                        # hides under the remaining key chunks.
                        if causal and j == 7:
                            pending.append(normalize_start(b, h, 0, y_acc[0]))
                        if causal and j == 10 and pending:
                            normalize_finish(pending.pop(0))
                    if not causal:
                        pending.append(normalize_start(b, h, 0, y_acc[0]))
                    pending.append(normalize_start(b, h, 1, y_acc[1]))
                    # Flush previous head's deferred finishes now that this
                    # head's scores give the chains time to land.
                    while len(pending) > (1 if (b, h) != (B - 1, QH - 1) else 0):
                        normalize_finish(pending.pop(0))
                    for thunk in deferred:
                        thunk()
                    deferred = []
                    if (b, h) == (B - 1, QH - 1):
                        while pending:
                            normalize_finish(pending.pop(0))
                    if h % 2 == 1:
                        bb, ii = b, h // 2
                        if (b, h) == (B - 1, QH - 1):
                            allgather(bb, ii)
                        else:
                            deferred.append(lambda bb=bb, ii=ii: allgather(bb, ii))
            for thunk in deferred:
                thunk()

        # ------------- phase 4: output projection (out^T shard) --------
        with tc.tile_pool(
            name="popool", bufs=1, space="PSUM"
        ) as pop, tc.tile_pool(name="osb", bufs=2) as osp:
            for b in range(B):
                pos = [
                    pop.tile([128, T], F32, name=f"po{m}", tag=f"po{m}")
                    for m in range(2)
                ]
                # k-chunks in AllGather arrival order: evens (head pair 0 of
                # each core) landed first, odds second.
                korder = [2 * g for g in range(NCORES)] + [
                    2 * g + 1 for g in range(NCORES)
                ]
                for idx, kc in enumerate(korder):
                    ysl = prefetched.pop((b, kc), None)
                    if ysl is None:
                        ysl = load_ysl(b, kc)
                    for m in range(2):
                        for u in range(0, T, 512):
                            nc.tensor.matmul(
                                pos[m][:, u : u + 512],
                                wo_sb[:, kc, m * 128 : (m + 1) * 128],
                                ysl[:, u : u + 512],
                                start=(idx == 0),
                                stop=(idx == KC - 1),
                            )
                for m in range(2):
                    osb = osp.tile([128, T], BF16, name="osb")
                    nc.vector.tensor_copy(osb[:], pos[m][:])
                    nc.gpsimd.dma_start(
                        outT[m, :, b * T : (b + 1) * T], osb[:]
                    )
    if compile:
        nc.compile()
    return nc


_CACHE: dict = {}


def _get_compiled(mode: str) -> bacc.Bacc:
    if mode not in _CACHE:
        _CACHE[mode] = _build(mode)
    return _CACHE[mode]


def _prep_inputs(x, attn_mask, Wq, Wk, Wv, Wo, mode):
    x = np.asarray(x, dtype=np.float32)
    Wq = np.asarray(Wq, dtype=np.float32) * 0.125  # fold 1/sqrt(64) into Wq
    Wk = np.asarray(Wk, dtype=np.float32)
    Wv = np.asarray(Wv, dtype=np.float32)
    Wo = np.asarray(Wo, dtype=np.float32)

    xT = (
        np.ascontiguousarray(x.transpose(0, 2, 1))
        .reshape(B, KC, 128, T)
        .astype(NPBF16)
    )
    maskT = None
    if mode == "general":
        mask2d = np.asarray(attn_mask, dtype=np.float32).reshape(T, T)
        maskT = np.stack(
            [mask2d[:, 128 * j : 128 * (j + 1)].T for j in range(JC)]
        ).astype(NPBF16)

    in_maps = []
    for c in range(NCORES):
        wq_c = np.ascontiguousarray(Wq[:, c * DQ : (c + 1) * DQ]).reshape(
            KC, 128, DQ
        ).astype(NPBF16)
        wkv_c = np.concatenate(
            [Wk[:, c * DH : (c + 1) * DH], Wv[:, c * DH : (c + 1) * DH]], axis=1
        ).reshape(KC, 128, 128).astype(NPBF16)
        wo_c = np.ascontiguousarray(Wo[:, c * DQ : (c + 1) * DQ]).reshape(
            KC, 128, DQ
        ).astype(NPBF16)
        im = {
            "xT": xT,
            "wq": wq_c,
            "wkv": wkv_c,
            "wo": wo_c,
            "ident": np.eye(128, dtype=NPBF16),
        }
        if maskT is not None:
            im["maskT"] = maskT
        in_maps.append(im)
    return in_maps


def _mask_mode(attn_mask) -> str:
    mask2d = np.asarray(attn_mask, dtype=np.float32).reshape(T, T)
    if not mask2d.any():
        return "zeros"
    ref = np.triu(np.full((T, T), -1e9, dtype=np.float32), k=1)
    if np.array_equal(mask2d, ref):
        return "causal"
    return "general"


def _run(x, attn_mask, Wq, Wk, Wv, Wo, trace=False, trace_cores=None):
    mode = _mask_mode(attn_mask)
    nc = _get_compiled(mode)
    in_maps = _prep_inputs(x, attn_mask, Wq, Wk, Wv, Wo, mode)
    res = run_bass_kernel_spmd(
        nc,
        in_maps,
        core_ids=list(range(NCORES)),
        trace=trace,
        trace_cores=trace_cores,
    )
    outT = np.concatenate(
        [
            np.asarray(r["outT"]).astype(np.float32).reshape(DQ, NT)
            for r in res.results
        ],
        axis=0,
    )
    out = np.ascontiguousarray(outT.T).reshape(B, T, HID).astype(np.float32)
    return out, res


def kernel(x, attn_mask, Wq, Wk, Wv, Wo):
    out, _ = _run(x, attn_mask, Wq, Wk, Wv, Wo)
    return out


if __name__ == "__main__":
    rng = np.random.default_rng(0)
    x = rng.standard_normal((B, T, HID), dtype=np.float32)
    mask = np.triu(np.full((T, T), -1e9, dtype=np.float32), k=1)[None, None]
    s = 1.0 / np.sqrt(HID)
    Wq = rng.standard_normal((HID, HEADS * DH), dtype=np.float32) * s
    Wk = rng.standard_normal((HID, KV_HEADS * DH), dtype=np.float32) * s
    Wv = rng.standard_normal((HID, KV_HEADS * DH), dtype=np.float32) * s
    Wo = rng.standard_normal((HEADS * DH, HID), dtype=np.float32) * s
    out = kernel(x, mask, Wq, Wk, Wv, Wo)
    print("out", out.shape, out.dtype, np.abs(out).mean())


# revision 26
# speedup vs baseline: 1.0209x; 1.0209x over previous
"""GQA attention (B=2, T=2048, HID=2048, 32 q-heads / 8 kv-heads, d=64)
distributed over 8 TRN2 NeuronCores.

Sharding: tensor-parallel over heads. Core c owns q-heads [4c, 4c+4) and
kv-head c (column shards of Wq/Wk/Wv), plus the matching column shard of Wo
used to compute out^T rows. x is replicated (host pre-transposes to [hid, tok]
and casts to bf16). After each head-pair the core AllGathers its y^T
[128, 2048] block; the out projection consumes k-chunks in arrival order and
emits outT[256c:256c+256, :] in bf16. The host concatenates and transposes.

Phase 2 pipeline: per key chunk j, scores land in [128, 1024] PSUM chunks
(double buffered), exp runs on ACT into a [128, 2048] pT tile, the causal
triangle is zeroed afterwards with affine_select on gpsimd (so no mask adds
and no mask input), and PV accumulates into two [65, 1024] y_acc halves that
free as soon as each half is normalized. The softmax denominator comes from a
ones-column appended to V; its reciprocal is computed at [128, 8] shape via a
DMA round-trip (reciprocal on a [1, 1024] row is ~15 us on DVE, the
transposed form is free).
"""

import os
import sys

import numpy as np

for _p in ("/opt/trn_rl_repo", "/root/.axon_site/_ro/trn_rl_repo"):
    if os.path.isdir(_p) and _p not in sys.path:
        sys.path.append(_p)

import ml_dtypes  # noqa: E402
from contextlib import ExitStack  # noqa: E402

import concourse.bass as bass  # noqa: E402
import concourse.tile as tile  # noqa: E402
from concourse import bacc, mybir  # noqa: E402
from concourse.bass_utils import run_bass_kernel_spmd  # noqa: E402

BF16 = mybir.dt.bfloat16
F32 = mybir.dt.float32
NPBF16 = ml_dtypes.bfloat16

B, T, HID = 2, 2048, 2048
NT = B * T
HEADS, KV_HEADS, DH = 32, 8, 64
NCORES = 8
QH = HEADS // NCORES          # q-heads per core
DQ = QH * DH                  # 256
KC = HID // 128               # 16 hidden-dim chunks
JC = T // 128                 # 16 key chunks of 128 per batch
HC = T // 1024                # 2 q-column halves of 1024 per batch
EXP = mybir.ActivationFunctionType.Exp


def _build(mode: str, debug: bool = False, compile: bool = True) -> bacc.Bacc:
    """mode: 'causal' (128-granular trim + affine triangle),
    'zeros' (no mask work), 'general' (additive mask from DRAM)."""
    causal = mode == "causal"
    nc = bacc.Bacc(
        "TRN2", target_bir_lowering=False, debug=debug, num_devices=NCORES
    )
    xT = nc.dram_tensor("xT", [B, KC, 128, T], BF16, kind="ExternalInput")
    wq = nc.dram_tensor("wq", [KC, 128, DQ], BF16, kind="ExternalInput")
    wkv = nc.dram_tensor("wkv", [KC, 128, 128], BF16, kind="ExternalInput")
    wo = nc.dram_tensor("wo", [KC, 128, DQ], BF16, kind="ExternalInput")
    ident = nc.dram_tensor("ident", [128, 128], BF16, kind="ExternalInput")
    if mode == "general":
        maskT = nc.dram_tensor("maskT", [JC, 128, T], BF16, kind="ExternalInput")
    outT = nc.dram_tensor("outT", [2, 128, NT], BF16, kind="ExternalOutput")

    def qlo(j):  # first valid q column for key chunk j
        return 128 * j if causal else 0

    with tile.TileContext(nc) as tc, ExitStack() as top:
        wpool = top.enter_context(tc.tile_pool(name="weights", bufs=1))
        wq_sb = wpool.tile([128, KC, DQ], BF16)
        wkv_sb = wpool.tile([128, KC, 128], BF16)
        nc.gpsimd.dma_start(wq_sb[:], wq[:, :, :].rearrange("k p d -> p k d"))
        nc.gpsimd.dma_start(wkv_sb[:], wkv[:, :, :].rearrange("k p d -> p k d"))

        qkv_pool = top.enter_context(tc.tile_pool(name="qkv", bufs=1))
        qT = [qkv_pool.tile([64, NT], BF16, name=f"qT{h}") for h in range(QH)]
        kT = qkv_pool.tile([64, NT], BF16, name="kT")
        vT = qkv_pool.tile([64, NT], BF16, name="vT")
        vones = qkv_pool.tile([128, B, JC, DH + 1], BF16, name="vones")
        yT_sb = [qkv_pool.tile([128, NT], BF16, name=f"yTsb{i}") for i in range(2)]
        ident_sb = wpool.tile([128, 128], BF16, name="ident_sb")
        ones_sb = wpool.tile([1, 64], BF16, name="ones_sb")
        nc.gpsimd.dma_start(ident_sb[:], ident[:])
        nc.vector.memset(ones_sb[:], 1.0)
        nc.vector.memset(vones[:, :, :, DH : DH + 1], 1.0)
        # 0/1 keep-mask for the causal in-block triangle: 1 iff qq >= kk
        tri01 = wpool.tile([128, 128], BF16, name="tri01")
        nc.gpsimd.memset(tri01[:], 1.0)
        nc.gpsimd.affine_select(
            out=tri01[:],
            in_=tri01[:],
            compare_op=mybir.AluOpType.is_ge,
            fill=0.0,
            base=0,
            pattern=[[1, 128]],
            channel_multiplier=-1,
        )

        # ---------------- phase 1: QKV projections (transposed layout) ------
        with tc.tile_pool(name="xcol", bufs=2) as xpool, tc.tile_pool(
            name="qkvps", bufs=3, space="PSUM"
        ) as qkvps:
            for n in range(B * 2):
                b, nn = divmod(n, 2)
                gcol = slice(b * T + nn * 1024, b * T + nn * 1024 + 1024)
                xc = xpool.tile([128, KC, 1024], BF16, name="xc")
                if n == 0:
                    # split the first load by k-chunk groups (contiguous
                    # free ranges) so the k-loop starts after ~1MB
                    for g in range(4):
                        nc.gpsimd.dma_start(
                            xc[:, 4 * g : 4 * g + 4, :],
                            xT[b, 4 * g : 4 * g + 4, :, 0:1024].rearrange(
                                "k p t -> p k t"
                            ),
                        )
                else:
                    nc.gpsimd.dma_start(
                        xc[:],
                        xT[b, :, :, nn * 1024 : (nn + 1) * 1024].rearrange(
                            "k p t -> p k t"
                        ),
                    )
                for m in range(2):  # q-head pairs (2m, 2m+1)
                    ps = qkvps.tile([128, 1024], F32, name="ps")
                    for u in (0, 512):  # matmul out must stay in one PSUM bank
                        for k in range(KC):
                            nc.tensor.matmul(
                                ps[:, u : u + 512],
                                wq_sb[:, k, m * 128 : (m + 1) * 128],
                                xc[:, k, u : u + 512],
                                start=(k == 0),
                                stop=(k == KC - 1),
                            )
                    nc.vector.tensor_copy(qT[2 * m][0:64, gcol], ps[0:64, :])
                    nc.vector.tensor_copy(qT[2 * m + 1][0:64, gcol], ps[64:128, :])
                ps = qkvps.tile([128, 1024], F32, name="ps")
                for u in (0, 512):
                    for k in range(KC):
                        nc.tensor.matmul(
                            ps[:, u : u + 512],
                            wkv_sb[:, k, :],
                            xc[:, k, u : u + 512],
                            start=(k == 0),
                            stop=(k == KC - 1),
                        )
                nc.vector.tensor_copy(kT[0:64, gcol], ps[0:64, :])
                nc.vector.tensor_copy(vT[0:64, gcol], ps[64:128, :])

        # ---------------- phase 1.5: V to natural layout (PE transpose) -----
        with tc.tile_pool(name="tps", bufs=3, space="PSUM") as tpool:
            for b in range(B):
                for j in range(JC):
                    tp = tpool.tile([128, DH], BF16, name="tp")
                    nc.tensor.transpose(
                        tp[:],
                        vT[0:64, b * T + j * 128 : b * T + (j + 1) * 128],
                        ident_sb[0:64, 0:64],
                    )
                    nc.vector.tensor_copy(vones[:, b, j, 0:DH], tp[:])

        # DRAM bounce buffers for the per-head-pair AllGathers.
        dpool = top.enter_context(tc.tile_pool(name="dram", bufs=1, space="DRAM"))
        yT_in = [
            [dpool.tile([128, T], BF16, name=f"yTin{b}_{i}") for i in range(2)]
            for b in range(B)
        ]
        yT_all = [
            [
                dpool.tile(
                    [NCORES, 128, T], BF16, addr_space="Shared", name=f"yTall{b}_{i}"
                )
                for i in range(2)
            ]
            for b in range(B)
        ]

        # Out-projection SBUF pools open before attention so wo and the first
        # y^T k-chunks prefetch while attention still runs.
        ylp = top.enter_context(tc.tile_pool(name="ysl", bufs=4))
        wop = top.enter_context(tc.tile_pool(name="wopool", bufs=1))
        wo_sb = wop.tile([128, KC, DQ], BF16, name="wo_sb")
        nc.gpsimd.dma_start(wo_sb[:], wo[:, :, :].rearrange("k p d -> p k d"))
        prefetched = {}

        def load_ysl(b, kc):
            t = ylp.tile([128, T], BF16, name="ysl")
            nc.gpsimd.dma_start(t[:], yT_all[b][kc % 2][kc // 2, :, :])
            return t

        # ---------------- phase 2: attention ------------------------------
        with tc.tile_pool(name="spool", bufs=2, space="PSUM") as spool, tc.tile_pool(
            name="ypool", bufs=1, space="PSUM"
        ) as ypsum, tc.tile_pool(name="ppool", bufs=3) as ppool, tc.tile_pool(
            name="npool", bufs=2
        ) as npool, (
            tc.tile_pool(name="mpool", bufs=1) if mode == "general" else ExitStack()
        ) as mpool:
            if mode == "general":
                mask_sb = mpool.tile([128, JC, T], BF16, name="mask_sb")
                nc.gpsimd.dma_start(
                    mask_sb[:], maskT[:, :, :].rearrange("j p w -> p j w")
                )

            def normalize_start(b, h, half, y_acc):
                """Copy den row out, kick the reciprocal round-trip. Returns
                state for normalize_finish. For half 1 also copy y to SBUF so
                the y_acc slot frees before the next head's first PV."""
                den_sb = npool.tile([1, 1024], F32, name="den_sb", tag="den")
                nc.vector.tensor_copy(den_sb[:], y_acc[DH : DH + 1, :])
                # copy y out of PSUM: frees the y_acc slot early AND the
                # final mul may only read one PSUM operand (rb_ps)
                yu = npool.tile([64, 1024], BF16, name="yu", tag="yu")
                nc.vector.tensor_copy(yu[:], y_acc[0:DH, :])
                den_t = npool.tile([128, 8], F32, name="den_t", tag="den_t")
                nc.gpsimd.dma_start(den_t[0:128, 0:8], den_sb[0:1, 0:1024])
                r_t = npool.tile([128, 8], F32, name="r_t", tag="r_t")
                nc.vector.reciprocal_approx_fast(r_t[:], den_t[:])
                r_row = npool.tile([1, 1024], BF16, name="r_row", tag="r_row")
                nc.gpsimd.dma_start(r_row[0:1, 0:1024], r_t[0:128, 0:8])
                return (b, h, half, y_acc, yu, r_row)

            def normalize_finish(state):
                b, h, half, y_acc, yu, r_row = state
                rb_ps = spool.tile([128, 1024], F32, name="rb_ps", tag="S")
                for u in (0, 512):
                    nc.tensor.matmul(
                        rb_ps[0:64, u : u + 512],
                        ones_sb[:],
                        r_row[0:1, u : u + 512],
                        start=True,
                        stop=True,
                    )
                dst = yT_sb[h // 2][
                    64 * (h % 2) : 64 * (h % 2) + 64,
                    b * T + half * 1024 : b * T + half * 1024 + 1024,
                ]
                nc.vector.tensor_mul(dst, yu[:], rb_ps[0:64, :])

            def allgather(b, i):
                nc.gpsimd.dma_start(
                    yT_in[b][i][:], yT_sb[i][:, b * T : (b + 1) * T]
                )
                nc.gpsimd.collective_compute(
                    "AllGather",
                    mybir.AluOpType.bypass,
                    replica_groups=[list(range(NCORES))],
                    ins=[yT_in[b][i].opt()],
                    outs=[yT_all[b][i].opt()],
                )
                # prefetch after AG2 fires: by then AG1 has completed, so the
                # in-order gpsimd queue won't block later den DMAs on it
                if (b, i) == (0, 1):
                    for kc in (0, 2, 4):
                        prefetched[(0, kc)] = load_ysl(0, kc)

            pending = []   # deferred normalize_finish / allgather thunks
            deferred = []
            for b in range(B):
                for h in range(QH):
                    y_acc = [
                        ypsum.tile([DH + 1, 1024], F32, name=f"yacc{c}", tag=f"yacc{c}")
                        for c in range(2)
                    ]
                    for j in range(JC):
                        q0 = qlo(j)
                        pT = ppool.tile([128, T], BF16, name="pT")
                        lk = kT[0:64, b * T + j * 128 : b * T + (j + 1) * 128]
                        for half in range(2):
                            lo, hi = half * 1024, half * 1024 + 1024
                            qs = max(lo, q0)
                            if qs >= hi:
                                continue
                            w = hi - qs
                            S = spool.tile([128, 1024], F32, name="S")
                            for p0 in range(0, w, 512):
                                pw = min(512, w - p0)
                                nc.tensor.matmul(
                                    S[:, p0 : p0 + pw],
                                    lk,
                                    qT[h][
                                        0:64,
                                        b * T + qs + p0 : b * T + qs + p0 + pw,
                                    ],
                                    start=True,
                                    stop=True,
                                )
                            if mode == "general":
                                nc.vector.tensor_add(
                                    S[:, 0:w], S[:, 0:w], mask_sb[:, j, qs:hi]
                                )
                            nc.scalar.activation(pT[:, qs:hi], S[:, 0:w], EXP)
                            if causal and qs == q0:
                                # zero the in-block triangle: keep col qq of
                                # partition kk iff qq >= kk
                                nc.vector.tensor_mul(
                                    pT[:, q0 : q0 + 128],
                                    pT[:, q0 : q0 + 128],
                                    tri01[:],
                                )
                        for half in range(2):
                            lo, hi = half * 1024, half * 1024 + 1024
                            qs = max(lo, q0)
                            if qs >= hi:
                                continue
                            a = qs - lo
                            bounds = [a, 512, 1024] if a < 512 else [a, 1024]
                            pieces = list(zip(bounds, bounds[1:]))
                            if qs == q0:
                                # diag piece last: its pT slice waits on the
                                # triangle zeroing; the other piece doesn't
                                pieces = pieces[::-1]
                            for pa, pb in pieces:
                                nc.tensor.matmul(
                                    y_acc[half][:, pa:pb],
                                    vones[:, b, j, :],
                                    pT[:, lo + pa : lo + pb],
                                    start=(j == 0),
                                    stop=(j == JC - 1 or (half == 0 and j == 7)),
                                    skip_group_check=True,
                                )
                        # half 0 complete once j==7 has accumulated (causal);
                        # start its normalization so the reciprocal round-trip
                        # hides under the remaining key chunks.
                        if causal and j == 7:
                            pending.append(normalize_start(b, h, 0, y_acc[0]))
                        if causal and j == 10 and pending:
                            normalize_finish(pending.pop(0))
                            # the AG deferred from the previous head-pair only
                            # needed that finish — fire it now, not at head end
                            for thunk in deferred:
                                thunk()
                            deferred = []
                    if not causal:
                        pending.append(normalize_start(b, h, 0, y_acc[0]))
                    pending.append(normalize_start(b, h, 1, y_acc[1]))
                    # Flush previous head's deferred finishes now that this
                    # head's scores give the chains time to land.
                    while len(pending) > (1 if (b, h) != (B - 1, QH - 1) else 0):
                        normalize_finish(pending.pop(0))
                    for thunk in deferred:
                        thunk()
                    deferred = []
                    if (b, h) == (B - 1, QH - 1):
                        while pending:
                            normalize_finish(pending.pop(0))
                    if h % 2 == 1:
                        bb, ii = b, h // 2
                        if (b, h) == (B - 1, QH - 1):
                            allgather(bb, ii)
                        else:
                            deferred.append(lambda bb=bb, ii=ii: allgather(bb, ii))
            for thunk in deferred:
                thunk()

        # ------------- phase 4: output projection (out^T shard) --------
        with tc.tile_pool(
            name="popool", bufs=1, space="PSUM"
        ) as pop, tc.tile_pool(name="osb", bufs=2) as osp:
            for b in range(B):
                pos = [
                    pop.tile([128, T], F32, name=f"po{m}", tag=f"po{m}")
                    for m in range(2)
                ]
                # k-chunks in AllGather arrival order: evens (head pair 0 of
                # each core) landed first, odds second.
                korder = [2 * g for g in range(NCORES)] + [
                    2 * g + 1 for g in range(NCORES)
                ]
                for idx, kc in enumerate(korder):
                    ysl = prefetched.pop((b, kc), None)
                    if ysl is None:
                        ysl = load_ysl(b, kc)
                    for m in range(2):
                        for u in range(0, T, 512):
                            nc.tensor.matmul(
                                pos[m][:, u : u + 512],
                                wo_sb[:, kc, m * 128 : (m + 1) * 128],
                                ysl[:, u : u + 512],
                                start=(idx == 0),
                                stop=(idx == KC - 1),
                            )
                for m in range(2):
                    osb = osp.tile([128, T], BF16, name="osb")
                    nc.vector.tensor_copy(osb[:], pos[m][:])
                    nc.gpsimd.dma_start(
                        outT[m, :, b * T : (b + 1) * T], osb[:]
                    )
    if compile:
        nc.compile()
    return nc


_CACHE: dict = {}


def _get_compiled(mode: str) -> bacc.Bacc:
    if mode not in _CACHE:
        _CACHE[mode] = _build(mode)
    return _CACHE[mode]


def _prep_inputs(x, attn_mask, Wq, Wk, Wv, Wo, mode):
    x = np.asarray(x, dtype=np.float32)
    Wq = np.asarray(Wq, dtype=np.float32) * 0.125  # fold 1/sqrt(64) into Wq
    Wk = np.asarray(Wk, dtype=np.float32)
    Wv = np.asarray(Wv, dtype=np.float32)
    Wo = np.asarray(Wo, dtype=np.float32)

    xT = (
        np.ascontiguousarray(x.transpose(0, 2, 1))
        .reshape(B, KC, 128, T)
        .astype(NPBF16)
    )
    maskT = None
    if mode == "general":
        mask2d = np.asarray(attn_mask, dtype=np.float32).reshape(T, T)
        maskT = np.stack(
            [mask2d[:, 128 * j : 128 * (j + 1)].T for j in range(JC)]
        ).astype(NPBF16)

    in_maps = []
    for c in range(NCORES):
        wq_c = np.ascontiguousarray(Wq[:, c * DQ : (c + 1) * DQ]).reshape(
            KC, 128, DQ
        ).astype(NPBF16)
        wkv_c = np.concatenate(
            [Wk[:, c * DH : (c + 1) * DH], Wv[:, c * DH : (c + 1) * DH]], axis=1
        ).reshape(KC, 128, 128).astype(NPBF16)
        wo_c = np.ascontiguousarray(Wo[:, c * DQ : (c + 1) * DQ]).reshape(
            KC, 128, DQ
        ).astype(NPBF16)
        im = {
            "xT": xT,
            "wq": wq_c,
            "wkv": wkv_c,
            "wo": wo_c,
            "ident": np.eye(128, dtype=NPBF16),
        }
        if maskT is not None:
            im["maskT"] = maskT
        in_maps.append(im)
    return in_maps


def _mask_mode(attn_mask) -> str:
    mask2d = np.asarray(attn_mask, dtype=np.float32).reshape(T, T)
    if not mask2d.any():
        return "zeros"
    ref = np.triu(np.full((T, T), -1e9, dtype=np.float32), k=1)
    if np.array_equal(mask2d, ref):
        return "causal"
    return "general"


def _run(x, attn_mask, Wq, Wk, Wv, Wo, trace=False, trace_cores=None):
    mode = _mask_mode(attn_mask)
    nc = _get_compiled(mode)
    in_maps = _prep_inputs(x, attn_mask, Wq, Wk, Wv, Wo, mode)
    res = run_bass_kernel_spmd(
        nc,
        in_maps,
        core_ids=list(range(NCORES)),
        trace=trace,
        trace_cores=trace_cores,
    )
    outT = np.concatenate(
        [
            np.asarray(r["outT"]).astype(np.float32).reshape(DQ, NT)
            for r in res.results
        ],
        axis=0,
    )
    out = np.ascontiguousarray(outT.T).reshape(B, T, HID).astype(np.float32)
    return out, res


def kernel(x, attn_mask, Wq, Wk, Wv, Wo):
    out, _ = _run(x, attn_mask, Wq, Wk, Wv, Wo)
    return out


if __name__ == "__main__":
    rng = np.random.default_rng(0)
    x = rng.standard_normal((B, T, HID), dtype=np.float32)
    mask = np.triu(np.full((T, T), -1e9, dtype=np.float32), k=1)[None, None]
    s = 1.0 / np.sqrt(HID)
    Wq = rng.standard_normal((HID, HEADS * DH), dtype=np.float32) * s
    Wk = rng.standard_normal((HID, KV_HEADS * DH), dtype=np.float32) * s
    Wv = rng.standard_normal((HID, KV_HEADS * DH), dtype=np.float32) * s
    Wo = rng.standard_normal((HEADS * DH, HID), dtype=np.float32) * s
    out = kernel(x, mask, Wq, Wk, Wv, Wo)
    print("out", out.shape, out.dtype, np.abs(out).mean())


# revision 29
# speedup vs baseline: 1.1703x; 1.1463x over previous
"""GQA attention (B=2, T=2048, HID=2048, 32 q-heads / 8 kv-heads, d=64)
distributed over 8 TRN2 NeuronCores.

Sharding: tensor-parallel over heads. Core c owns q-heads [4c, 4c+4) and
kv-head c (column shards of Wq/Wk/Wv), plus the matching column shard of Wo
used to compute out^T rows. x is replicated (host pre-transposes to [hid, tok]
and casts to bf16). After each head-pair the core AllGathers its y^T
[128, 2048] block; the out projection consumes k-chunks in arrival order and
emits outT[256c:256c+256, :] in bf16. The host concatenates and transposes.

Phase 2 pipeline: per key chunk j, scores land in [128, 1024] PSUM chunks
(double buffered), exp runs on ACT into a [128, 2048] pT tile, the causal
triangle is zeroed afterwards with affine_select on gpsimd (so no mask adds
and no mask input), and PV accumulates into two [65, 1024] y_acc halves that
free as soon as each half is normalized. The softmax denominator comes from a
ones-column appended to V; its reciprocal is computed at [128, 8] shape via a
DMA round-trip (reciprocal on a [1, 1024] row is ~15 us on DVE, the
transposed form is free).
"""

import os
import sys

import numpy as np

for _p in ("/opt/trn_rl_repo", "/root/.axon_site/_ro/trn_rl_repo"):
    if os.path.isdir(_p) and _p not in sys.path:
        sys.path.append(_p)

import ml_dtypes  # noqa: E402
from contextlib import ExitStack  # noqa: E402

import concourse.bass as bass  # noqa: E402
import concourse.tile as tile  # noqa: E402
from concourse import bacc, mybir  # noqa: E402
from concourse.bass_utils import run_bass_kernel_spmd  # noqa: E402

BF16 = mybir.dt.bfloat16
F32 = mybir.dt.float32
NPBF16 = ml_dtypes.bfloat16

B, T, HID = 2, 2048, 2048
NT = B * T
HEADS, KV_HEADS, DH = 32, 8, 64
NCORES = 8
QH = HEADS // NCORES          # q-heads per core
DQ = QH * DH                  # 256
KC = HID // 128               # 16 hidden-dim chunks
JC = T // 128                 # 16 key chunks of 128 per batch
HC = T // 1024                # 2 q-column halves of 1024 per batch
EXP = mybir.ActivationFunctionType.Exp


def _build(mode: str, debug: bool = False, compile: bool = True) -> bacc.Bacc:
    """mode: 'causal' (128-granular trim + affine triangle),
    'zeros' (no mask work), 'general' (additive mask from DRAM)."""
    causal = mode == "causal"
    nc = bacc.Bacc(
        "TRN2", target_bir_lowering=False, debug=debug, num_devices=NCORES
    )
    xT = nc.dram_tensor("xT", [B, KC, 128, T], BF16, kind="ExternalInput")
    wq = nc.dram_tensor("wq", [KC, 128, DQ], BF16, kind="ExternalInput")
    wkv = nc.dram_tensor("wkv", [KC, 128, 128], BF16, kind="ExternalInput")
    wo = nc.dram_tensor("wo", [KC, 128, DQ], BF16, kind="ExternalInput")
    ident = nc.dram_tensor("ident", [128, 128], BF16, kind="ExternalInput")
    if mode == "general":
        maskT = nc.dram_tensor("maskT", [JC, 128, T], BF16, kind="ExternalInput")
    outT = nc.dram_tensor("outT", [2, 128, NT], BF16, kind="ExternalOutput")

    def qlo(j):  # first valid q column for key chunk j
        return 128 * j if causal else 0

    with tile.TileContext(nc) as tc, ExitStack() as top:
        wpool = top.enter_context(tc.tile_pool(name="weights", bufs=1))
        wq_sb = wpool.tile([128, KC, DQ], BF16)
        wkv_sb = wpool.tile([128, KC, 128], BF16)
        # staggered k-group loads: the first projection matmuls only need
        # the leading k-chunks, so compute starts before the full load lands
        for g in range(4):
            nc.gpsimd.dma_start(
                wq_sb[:, 4 * g : 4 * g + 4, :],
                wq[4 * g : 4 * g + 4, :, :].rearrange("k p d -> p k d"),
            )
        nc.gpsimd.dma_start(wkv_sb[:], wkv[:, :, :].rearrange("k p d -> p k d"))

        qkv_pool = top.enter_context(tc.tile_pool(name="qkv", bufs=1))
        qT = [qkv_pool.tile([64, NT], BF16, name=f"qT{h}") for h in range(QH)]
        kT = qkv_pool.tile([64, NT], BF16, name="kT")
        vT = qkv_pool.tile([64, NT], BF16, name="vT")
        vones = qkv_pool.tile([128, B, JC, DH + 1], BF16, name="vones")
        yT_sb = [qkv_pool.tile([128, NT], BF16, name=f"yTsb{i}") for i in range(2)]
        ident_sb = wpool.tile([128, 128], BF16, name="ident_sb")
        ones_sb = wpool.tile([1, 64], BF16, name="ones_sb")
        nc.gpsimd.dma_start(ident_sb[:], ident[:])
        nc.vector.memset(ones_sb[:], 1.0)
        nc.vector.memset(vones[:, :, :, DH : DH + 1], 1.0)
        # 0/1 keep-mask for the causal in-block triangle: 1 iff qq >= kk
        tri01 = wpool.tile([128, 128], BF16, name="tri01")
        nc.gpsimd.memset(tri01[:], 1.0)
        nc.gpsimd.affine_select(
            out=tri01[:],
            in_=tri01[:],
            compare_op=mybir.AluOpType.is_ge,
            fill=0.0,
            base=0,
            pattern=[[1, 128]],
            channel_multiplier=-1,
        )

        # ---------------- phase 1: QKV projections (transposed layout) ------
        with tc.tile_pool(name="xcol", bufs=2) as xpool, tc.tile_pool(
            name="qkvps", bufs=3, space="PSUM"
        ) as qkvps:
            for n in range(B * 2):
                b, nn = divmod(n, 2)
                gcol = slice(b * T + nn * 1024, b * T + nn * 1024 + 1024)
                xc = xpool.tile([128, KC, 1024], BF16, name="xc")
                if n == 0:
                    # split the first load by k-chunk groups (contiguous
                    # free ranges) so the k-loop starts after ~1MB
                    for g in range(4):
                        nc.gpsimd.dma_start(
                            xc[:, 4 * g : 4 * g + 4, :],
                            xT[b, 4 * g : 4 * g + 4, :, 0:1024].rearrange(
                                "k p t -> p k t"
                            ),
                        )
                else:
                    nc.gpsimd.dma_start(
                        xc[:],
                        xT[b, :, :, nn * 1024 : (nn + 1) * 1024].rearrange(
                            "k p t -> p k t"
                        ),
                    )
                for m in range(2):  # q-head pairs (2m, 2m+1)
                    ps = qkvps.tile([128, 1024], F32, name="ps")
                    for u in (0, 512):  # matmul out must stay in one PSUM bank
                        for k in range(KC):
                            nc.tensor.matmul(
                                ps[:, u : u + 512],
                                wq_sb[:, k, m * 128 : (m + 1) * 128],
                                xc[:, k, u : u + 512],
                                start=(k == 0),
                                stop=(k == KC - 1),
                            )
                    nc.vector.tensor_copy(qT[2 * m][0:64, gcol], ps[0:64, :])
                    nc.vector.tensor_copy(qT[2 * m + 1][0:64, gcol], ps[64:128, :])
                ps = qkvps.tile([128, 1024], F32, name="ps")
                for u in (0, 512):
                    for k in range(KC):
                        nc.tensor.matmul(
                            ps[:, u : u + 512],
                            wkv_sb[:, k, :],
                            xc[:, k, u : u + 512],
                            start=(k == 0),
                            stop=(k == KC - 1),
                        )
                nc.vector.tensor_copy(kT[0:64, gcol], ps[0:64, :])
                nc.vector.tensor_copy(vT[0:64, gcol], ps[64:128, :])

        # ---------------- phase 1.5: V to natural layout (PE transpose) -----
        with tc.tile_pool(name="tps", bufs=3, space="PSUM") as tpool:
            for b in range(B):
                for j in range(JC):
                    tp = tpool.tile([128, DH], BF16, name="tp")
                    nc.tensor.transpose(
                        tp[:],
                        vT[0:64, b * T + j * 128 : b * T + (j + 1) * 128],
                        ident_sb[0:64, 0:64],
                    )
                    nc.vector.tensor_copy(vones[:, b, j, 0:DH], tp[:])

        # DRAM bounce buffers for the per-head-pair AllGathers.
        dpool = top.enter_context(tc.tile_pool(name="dram", bufs=1, space="DRAM"))
        yT_in = [
            [dpool.tile([128, T], BF16, name=f"yTin{b}_{i}") for i in range(2)]
            for b in range(B)
        ]
        yT_all = [
            [
                dpool.tile(
                    [NCORES, 128, T], BF16, addr_space="Shared", name=f"yTall{b}_{i}"
                )
                for i in range(2)
            ]
            for b in range(B)
        ]

        # Out-projection SBUF pools open before attention so wo and the first
        # y^T k-chunks prefetch while attention still runs.
        ylp = top.enter_context(tc.tile_pool(name="ysl", bufs=4))
        wop = top.enter_context(tc.tile_pool(name="wopool", bufs=1))
        wo_sb = wop.tile([128, KC, DQ], BF16, name="wo_sb")
        nc.gpsimd.dma_start(wo_sb[:], wo[:, :, :].rearrange("k p d -> p k d"))
        prefetched = {}

        def load_ysl(b, kc):
            t = ylp.tile([128, T], BF16, name="ysl")
            nc.gpsimd.dma_start(t[:], yT_all[b][kc % 2][kc // 2, :, :])
            return t

        # ---------------- phase 2: attention ------------------------------
        with tc.tile_pool(name="spool", bufs=2, space="PSUM") as spool, tc.tile_pool(
            name="ypool", bufs=1, space="PSUM"
        ) as ypsum, tc.tile_pool(name="ppool", bufs=3) as ppool, tc.tile_pool(
            name="npool", bufs=2
        ) as npool, (
            tc.tile_pool(name="mpool", bufs=1) if mode == "general" else ExitStack()
        ) as mpool:
            if mode == "general":
                mask_sb = mpool.tile([128, JC, T], BF16, name="mask_sb")
                nc.gpsimd.dma_start(
                    mask_sb[:], maskT[:, :, :].rearrange("j p w -> p j w")
                )

            def normalize_start(b, h, half, y_acc):
                """Copy den row out, kick the reciprocal round-trip. Returns
                state for normalize_finish. For half 1 also copy y to SBUF so
                the y_acc slot frees before the next head's first PV."""
                den_sb = npool.tile([1, 1024], F32, name="den_sb", tag="den")
                nc.vector.tensor_copy(den_sb[:], y_acc[DH : DH + 1, :])
                # copy y out of PSUM: frees the y_acc slot early AND the
                # final mul may only read one PSUM operand (rb_ps)
                yu = npool.tile([64, 1024], BF16, name="yu", tag="yu")
                nc.vector.tensor_copy(yu[:], y_acc[0:DH, :])
                # DMA round-trips ride the DVE's own HWDGE queue so they never
                # queue behind AllGather triggers / bulk loads on gpsimd
                den_t = npool.tile([128, 8], F32, name="den_t", tag="den_t")
                nc.scalar.dma_start(den_t[0:128, 0:8], den_sb[0:1, 0:1024])
                r_t = npool.tile([128, 8], F32, name="r_t", tag="r_t")
                nc.vector.reciprocal_approx_fast(r_t[:], den_t[:])
                r_tb = npool.tile([128, 8], BF16, name="r_tb", tag="r_tb")
                nc.vector.tensor_copy(r_tb[:], r_t[:])
                r_row = npool.tile([1, 1024], BF16, name="r_row", tag="r_row")
                nc.scalar.dma_start(r_row[0:1, 0:1024], r_tb[0:128, 0:8])
                return (b, h, half, y_acc, yu, r_row)

            def normalize_finish(state):
                b, h, half, y_acc, yu, r_row = state
                rb_ps = spool.tile([128, 1024], F32, name="rb_ps", tag="S")
                for u in (0, 512):
                    nc.tensor.matmul(
                        rb_ps[0:64, u : u + 512],
                        ones_sb[:],
                        r_row[0:1, u : u + 512],
                        start=True,
                        stop=True,
                    )
                dst = yT_sb[h // 2][
                    64 * (h % 2) : 64 * (h % 2) + 64,
                    b * T + half * 1024 : b * T + half * 1024 + 1024,
                ]
                nc.vector.tensor_mul(dst, yu[:], rb_ps[0:64, :])

            def allgather(b, i):
                nc.gpsimd.dma_start(
                    yT_in[b][i][:], yT_sb[i][:, b * T : (b + 1) * T]
                )
                nc.gpsimd.collective_compute(
                    "AllGather",
                    mybir.AluOpType.bypass,
                    replica_groups=[list(range(NCORES))],
                    ins=[yT_in[b][i].opt()],
                    outs=[yT_all[b][i].opt()],
                )
                # prefetch after AG2 fires: by then AG1 has completed, so the
                # in-order gpsimd queue won't block later den DMAs on it
                if (b, i) == (0, 1):
                    for kc in (0, 2, 4):
                        prefetched[(0, kc)] = load_ysl(0, kc)

            pending = []   # deferred normalize_finish / allgather thunks
            deferred = []
            for b in range(B):
                for h in range(QH):
                    y_acc = [
                        ypsum.tile([DH + 1, 1024], F32, name=f"yacc{c}", tag=f"yacc{c}")
                        for c in range(2)
                    ]
                    for j in range(JC):
                        q0 = qlo(j)
                        pT = ppool.tile([128, T], BF16, name="pT")
                        lk = kT[0:64, b * T + j * 128 : b * T + (j + 1) * 128]
                        for half in range(2):
                            lo, hi = half * 1024, half * 1024 + 1024
                            qs = max(lo, q0)
                            if qs >= hi:
                                continue
                            w = hi - qs
                            S = spool.tile([128, 1024], F32, name="S")
                            for p0 in range(0, w, 512):
                                pw = min(512, w - p0)
                                nc.tensor.matmul(
                                    S[:, p0 : p0 + pw],
                                    lk,
                                    qT[h][
                                        0:64,
                                        b * T + qs + p0 : b * T + qs + p0 + pw,
                                    ],
                                    start=True,
                                    stop=True,
                                )
                            if mode == "general":
                                nc.vector.tensor_add(
                                    S[:, 0:w], S[:, 0:w], mask_sb[:, j, qs:hi]
                                )
                            nc.scalar.activation(pT[:, qs:hi], S[:, 0:w], EXP)
                            if causal and qs == q0:
                                # zero the in-block triangle: keep col qq of
                                # partition kk iff qq >= kk
                                nc.vector.tensor_mul(
                                    pT[:, q0 : q0 + 128],
                                    pT[:, q0 : q0 + 128],
                                    tri01[:],
                                )
                        for half in range(2):
                            lo, hi = half * 1024, half * 1024 + 1024
                            qs = max(lo, q0)
                            if qs >= hi:
                                continue
                            a = qs - lo
                            bounds = [a, 512, 1024] if a < 512 else [a, 1024]
                            pieces = list(zip(bounds, bounds[1:]))
                            if qs == q0:
                                # diag piece last: its pT slice waits on the
                                # triangle zeroing; the other piece doesn't
                                pieces = pieces[::-1]
                            for pa, pb in pieces:
                                nc.tensor.matmul(
                                    y_acc[half][:, pa:pb],
                                    vones[:, b, j, :],
                                    pT[:, lo + pa : lo + pb],
                                    start=(j == 0),
                                    stop=(j == JC - 1 or (half == 0 and j == 7)),
                                    skip_group_check=True,
                                )
                        # half 0 complete once j==7 has accumulated (causal);
                        # start its normalization so the reciprocal round-trip
                        # hides under the remaining key chunks.
                        if causal and j == 7:
                            pending.append(normalize_start(b, h, 0, y_acc[0]))
                        if causal and j == 10 and pending:
                            normalize_finish(pending.pop(0))
                            # the AG deferred from the previous head-pair only
                            # needed that finish — fire it now, not at head end
                            for thunk in deferred:
                                thunk()
                            deferred = []
                    if not causal:
                        pending.append(normalize_start(b, h, 0, y_acc[0]))
                    pending.append(normalize_start(b, h, 1, y_acc[1]))
                    # Flush previous head's deferred finishes now that this
                    # head's scores give the chains time to land.
                    while len(pending) > (1 if (b, h) != (B - 1, QH - 1) else 0):
                        normalize_finish(pending.pop(0))
                    for thunk in deferred:
                        thunk()
                    deferred = []
                    if (b, h) == (B - 1, QH - 1):
                        while pending:
                            normalize_finish(pending.pop(0))
                    if h % 2 == 1:
                        bb, ii = b, h // 2
                        if (b, h) == (B - 1, QH - 1):
                            allgather(bb, ii)
                        else:
                            deferred.append(lambda bb=bb, ii=ii: allgather(bb, ii))
            for thunk in deferred:
                thunk()

        # ------------- phase 4: output projection (out^T shard) --------
        with tc.tile_pool(
            name="popool", bufs=1, space="PSUM"
        ) as pop, tc.tile_pool(name="osb", bufs=2) as osp:
            for b in range(B):
                pos = [
                    pop.tile([128, T], F32, name=f"po{m}", tag=f"po{m}")
                    for m in range(2)
                ]
                # k-chunks in AllGather arrival order: evens (head pair 0 of
                # each core) landed first, odds second.
                korder = [2 * g for g in range(NCORES)] + [
                    2 * g + 1 for g in range(NCORES)
                ]
                for idx, kc in enumerate(korder):
                    ysl = prefetched.pop((b, kc), None)
                    if ysl is None:
                        ysl = load_ysl(b, kc)
                    for m in range(2):
                        for u in range(0, T, 512):
                            nc.tensor.matmul(
                                pos[m][:, u : u + 512],
                                wo_sb[:, kc, m * 128 : (m + 1) * 128],
                                ysl[:, u : u + 512],
                                start=(idx == 0),
                                stop=(idx == KC - 1),
                            )
                for m in range(2):
                    osb = osp.tile([128, T], BF16, name="osb")
                    nc.vector.tensor_copy(osb[:], pos[m][:])
                    nc.gpsimd.dma_start(
                        outT[m, :, b * T : (b + 1) * T], osb[:]
                    )
    if compile:
        nc.compile()
    return nc


_CACHE: dict = {}


def _get_compiled(mode: str) -> bacc.Bacc:
    if mode not in _CACHE:
        _CACHE[mode] = _build(mode)
    return _CACHE[mode]


def _prep_inputs(x, attn_mask, Wq, Wk, Wv, Wo, mode):
    x = np.asarray(x, dtype=np.float32)
    Wq = np.asarray(Wq, dtype=np.float32) * 0.125  # fold 1/sqrt(64) into Wq
    Wk = np.asarray(Wk, dtype=np.float32)
    Wv = np.asarray(Wv, dtype=np.float32)
    Wo = np.asarray(Wo, dtype=np.float32)

    xT = (
        np.ascontiguousarray(x.transpose(0, 2, 1))
        .reshape(B, KC, 128, T)
        .astype(NPBF16)
    )
    maskT = None
    if mode == "general":
        mask2d = np.asarray(attn_mask, dtype=np.float32).reshape(T, T)
        maskT = np.stack(
            [mask2d[:, 128 * j : 128 * (j + 1)].T for j in range(JC)]
        ).astype(NPBF16)

    in_maps = []
    for c in range(NCORES):
        wq_c = np.ascontiguousarray(Wq[:, c * DQ : (c + 1) * DQ]).reshape(
            KC, 128, DQ
        ).astype(NPBF16)
        wkv_c = np.concatenate(
            [Wk[:, c * DH : (c + 1) * DH], Wv[:, c * DH : (c + 1) * DH]], axis=1
        ).reshape(KC, 128, 128).astype(NPBF16)
        wo_c = np.ascontiguousarray(Wo[:, c * DQ : (c + 1) * DQ]).reshape(
            KC, 128, DQ
        ).astype(NPBF16)
        im = {
            "xT": xT,
            "wq": wq_c,
            "wkv": wkv_c,
            "wo": wo_c,
            "ident": np.eye(128, dtype=NPBF16),
        }
        if maskT is not None:
            im["maskT"] = maskT
        in_maps.append(im)
    return in_maps


def _mask_mode(attn_mask) -> str:
    mask2d = np.asarray(attn_mask, dtype=np.float32).reshape(T, T)
    if not mask2d.any():
        return "zeros"
    ref = np.triu(np.full((T, T), -1e9, dtype=np.float32), k=1)
    if np.array_equal(mask2d, ref):
        return "causal"
    return "general"


def _run(x, attn_mask, Wq, Wk, Wv, Wo, trace=False, trace_cores=None):
    mode = _mask_mode(attn_mask)
    nc = _get_compiled(mode)
    in_maps = _prep_inputs(x, attn_mask, Wq, Wk, Wv, Wo, mode)
    res = run_bass_kernel_spmd(
        nc,
        in_maps,
        core_ids=list(range(NCORES)),
        trace=trace,
        trace_cores=trace_cores,
    )
    outT = np.concatenate(
        [
            np.asarray(r["outT"]).astype(np.float32).reshape(DQ, NT)
            for r in res.results
        ],
        axis=0,
    )
    out = np.ascontiguousarray(outT.T).reshape(B, T, HID).astype(np.float32)
    return out, res


def kernel(x, attn_mask, Wq, Wk, Wv, Wo):
    out, _ = _run(x, attn_mask, Wq, Wk, Wv, Wo)
    return out


if __name__ == "__main__":
    rng = np.random.default_rng(0)
    x = rng.standard_normal((B, T, HID), dtype=np.float32)
    mask = np.triu(np.full((T, T), -1e9, dtype=np.float32), k=1)[None, None]
    s = 1.0 / np.sqrt(HID)
    Wq = rng.standard_normal((HID, HEADS * DH), dtype=np.float32) * s
    Wk = rng.standard_normal((HID, KV_HEADS * DH), dtype=np.float32) * s
    Wv = rng.standard_normal((HID, KV_HEADS * DH), dtype=np.float32) * s
    Wo = rng.standard_normal((HEADS * DH, HID), dtype=np.float32) * s
    out = kernel(x, mask, Wq, Wk, Wv, Wo)
    print("out", out.shape, out.dtype, np.abs(out).mean())


# revision 31
# speedup vs baseline: 1.1869x; 1.0142x over previous
"""GQA attention (B=2, T=2048, HID=2048, 32 q-heads / 8 kv-heads, d=64)
distributed over 8 TRN2 NeuronCores.

Sharding: tensor-parallel over heads. Core c owns q-heads [4c, 4c+4) and
kv-head c (column shards of Wq/Wk/Wv), plus the matching column shard of Wo
used to compute out^T rows. x is replicated (host pre-transposes to [hid, tok]
and casts to bf16). After each head-pair the core AllGathers its y^T
[128, 2048] block; the out projection consumes k-chunks in arrival order and
emits outT[256c:256c+256, :] in bf16. The host concatenates and transposes.

Phase 2 pipeline: per key chunk j, scores land in [128, 1024] PSUM chunks
(double buffered), exp runs on ACT into a [128, 2048] pT tile, the causal
triangle is zeroed afterwards with affine_select on gpsimd (so no mask adds
and no mask input), and PV accumulates into two [65, 1024] y_acc halves that
free as soon as each half is normalized. The softmax denominator comes from a
ones-column appended to V; its reciprocal is computed at [128, 8] shape via a
DMA round-trip (reciprocal on a [1, 1024] row is ~15 us on DVE, the
transposed form is free).
"""

import os
import sys

import numpy as np

for _p in ("/opt/trn_rl_repo", "/root/.axon_site/_ro/trn_rl_repo"):
    if os.path.isdir(_p) and _p not in sys.path:
        sys.path.append(_p)

import ml_dtypes  # noqa: E402
from contextlib import ExitStack  # noqa: E402

import concourse.bass as bass  # noqa: E402
import concourse.tile as tile  # noqa: E402
from concourse import bacc, mybir  # noqa: E402
from concourse.bass_utils import run_bass_kernel_spmd  # noqa: E402

BF16 = mybir.dt.bfloat16
F32 = mybir.dt.float32
NPBF16 = ml_dtypes.bfloat16

B, T, HID = 2, 2048, 2048
NT = B * T
HEADS, KV_HEADS, DH = 32, 8, 64
NCORES = 8
QH = HEADS // NCORES          # q-heads per core
DQ = QH * DH                  # 256
KC = HID // 128               # 16 hidden-dim chunks
JC = T // 128                 # 16 key chunks of 128 per batch
HC = T // 1024                # 2 q-column halves of 1024 per batch
EXP = mybir.ActivationFunctionType.Exp


def _build(mode: str, debug: bool = False, compile: bool = True) -> bacc.Bacc:
    """mode: 'causal' (128-granular trim + affine triangle),
    'zeros' (no mask work), 'general' (additive mask from DRAM)."""
    causal = mode == "causal"
    nc = bacc.Bacc(
        "TRN2", target_bir_lowering=False, debug=debug, num_devices=NCORES
    )
    xT = nc.dram_tensor("xT", [B, KC, 128, T], BF16, kind="ExternalInput")
    wq = nc.dram_tensor("wq", [KC, 128, DQ], BF16, kind="ExternalInput")
    wkv = nc.dram_tensor("wkv", [KC, 128, 128], BF16, kind="ExternalInput")
    wo = nc.dram_tensor("wo", [KC, 128, DQ], BF16, kind="ExternalInput")
    ident = nc.dram_tensor("ident", [128, 128], BF16, kind="ExternalInput")
    if mode == "general":
        maskT = nc.dram_tensor("maskT", [JC, 128, T], BF16, kind="ExternalInput")
    outT = nc.dram_tensor("outT", [2, 128, NT], BF16, kind="ExternalOutput")

    def qlo(j):  # first valid q column for key chunk j
        return 128 * j if causal else 0

    with tile.TileContext(nc) as tc, ExitStack() as top:
        wpool = top.enter_context(tc.tile_pool(name="weights", bufs=1))
        wq_sb = wpool.tile([128, KC, DQ], BF16)
        wkv_sb = wpool.tile([128, KC, 128], BF16)
        # staggered k-group loads: the first projection matmuls only need
        # the leading k-chunks, so compute starts before the full load lands
        def load_wq_group(g):
            nc.gpsimd.dma_start(
                wq_sb[:, 4 * g : 4 * g + 4, :],
                wq[4 * g : 4 * g + 4, :, :].rearrange("k p d -> p k d"),
            )

        load_wq_group(0)

        qkv_pool = top.enter_context(tc.tile_pool(name="qkv", bufs=1))
        qT = [qkv_pool.tile([64, NT], BF16, name=f"qT{h}") for h in range(QH)]
        kT = qkv_pool.tile([64, NT], BF16, name="kT")
        vT = qkv_pool.tile([64, NT], BF16, name="vT")
        vones = qkv_pool.tile([128, B, JC, DH + 1], BF16, name="vones")
        yT_sb = [qkv_pool.tile([128, NT], BF16, name=f"yTsb{i}") for i in range(2)]
        ident_sb = wpool.tile([128, 128], BF16, name="ident_sb")
        ones_sb = wpool.tile([1, 64], BF16, name="ones_sb")
        nc.gpsimd.dma_start(ident_sb[:], ident[:])
        nc.vector.memset(ones_sb[:], 1.0)
        nc.vector.memset(vones[:, :, :, DH : DH + 1], 1.0)
        # 0/1 keep-mask for the causal in-block triangle: 1 iff qq >= kk
        tri01 = wpool.tile([128, 128], BF16, name="tri01")
        nc.gpsimd.memset(tri01[:], 1.0)
        nc.gpsimd.affine_select(
            out=tri01[:],
            in_=tri01[:],
            compare_op=mybir.AluOpType.is_ge,
            fill=0.0,
            base=0,
            pattern=[[1, 128]],
            channel_multiplier=-1,
        )

        # ---------------- phase 1: QKV projections (transposed layout) ------
        with tc.tile_pool(name="xcol", bufs=2) as xpool, tc.tile_pool(
            name="qkvps", bufs=3, space="PSUM"
        ) as qkvps:
            for n in range(B * 2):
                b, nn = divmod(n, 2)
                gcol = slice(b * T + nn * 1024, b * T + nn * 1024 + 1024)
                xc = xpool.tile([128, KC, 1024], BF16, name="xc")
                if n == 0:
                    # split the first load by k-chunk groups (contiguous
                    # free ranges), interleaved with the remaining weight
                    # groups, so the k-loop starts after ~2MB of DMA
                    for g in range(4):
                        nc.gpsimd.dma_start(
                            xc[:, 4 * g : 4 * g + 4, :],
                            xT[b, 4 * g : 4 * g + 4, :, 0:1024].rearrange(
                                "k p t -> p k t"
                            ),
                        )
                        if g < 3:
                            load_wq_group(g + 1)
                    nc.gpsimd.dma_start(
                        wkv_sb[:], wkv[:, :, :].rearrange("k p d -> p k d")
                    )
                else:
                    nc.gpsimd.dma_start(
                        xc[:],
                        xT[b, :, :, nn * 1024 : (nn + 1) * 1024].rearrange(
                            "k p t -> p k t"
                        ),
                    )
                for m in range(2):  # q-head pairs (2m, 2m+1)
                    ps = qkvps.tile([128, 1024], F32, name="ps")
                    for u in (0, 512):  # matmul out must stay in one PSUM bank
                        for k in range(KC):
                            nc.tensor.matmul(
                                ps[:, u : u + 512],
                                wq_sb[:, k, m * 128 : (m + 1) * 128],
                                xc[:, k, u : u + 512],
                                start=(k == 0),
                                stop=(k == KC - 1),
                            )
                    nc.vector.tensor_copy(qT[2 * m][0:64, gcol], ps[0:64, :])
                    nc.vector.tensor_copy(qT[2 * m + 1][0:64, gcol], ps[64:128, :])
                ps = qkvps.tile([128, 1024], F32, name="ps")
                for u in (0, 512):
                    for k in range(KC):
                        nc.tensor.matmul(
                            ps[:, u : u + 512],
                            wkv_sb[:, k, :],
                            xc[:, k, u : u + 512],
                            start=(k == 0),
                            stop=(k == KC - 1),
                        )
                nc.vector.tensor_copy(kT[0:64, gcol], ps[0:64, :])
                nc.vector.tensor_copy(vT[0:64, gcol], ps[64:128, :])

        # ---------------- phase 1.5: V to natural layout (PE transpose) -----
        with tc.tile_pool(name="tps", bufs=3, space="PSUM") as tpool:
            for b in range(B):
                for j in range(JC):
                    tp = tpool.tile([128, DH], BF16, name="tp")
                    nc.tensor.transpose(
                        tp[:],
                        vT[0:64, b * T + j * 128 : b * T + (j + 1) * 128],
                        ident_sb[0:64, 0:64],
                    )
                    nc.vector.tensor_copy(vones[:, b, j, 0:DH], tp[:])

        # DRAM bounce buffers for the per-head-pair AllGathers.
        dpool = top.enter_context(tc.tile_pool(name="dram", bufs=1, space="DRAM"))
        yT_in = [
            [dpool.tile([128, T], BF16, name=f"yTin{b}_{i}") for i in range(2)]
            for b in range(B)
        ]
        yT_all = [
            [
                dpool.tile(
                    [NCORES, 128, T], BF16, addr_space="Shared", name=f"yTall{b}_{i}"
                )
                for i in range(2)
            ]
            for b in range(B)
        ]

        # Out-projection SBUF pools open before attention so wo and the first
        # y^T k-chunks prefetch while attention still runs.
        ylp = top.enter_context(tc.tile_pool(name="ysl", bufs=4))
        wop = top.enter_context(tc.tile_pool(name="wopool", bufs=1))
        wo_sb = wop.tile([128, KC, DQ], BF16, name="wo_sb")
        nc.gpsimd.dma_start(wo_sb[:], wo[:, :, :].rearrange("k p d -> p k d"))
        prefetched = {}

        def load_ysl(b, kc):
            t = ylp.tile([128, T], BF16, name="ysl")
            nc.gpsimd.dma_start(t[:], yT_all[b][kc % 2][kc // 2, :, :])
            return t

        # ---------------- phase 2: attention ------------------------------
        with tc.tile_pool(name="spool", bufs=2, space="PSUM") as spool, tc.tile_pool(
            name="ypool", bufs=1, space="PSUM"
        ) as ypsum, tc.tile_pool(name="ppool", bufs=3) as ppool, tc.tile_pool(
            name="npool", bufs=2
        ) as npool, (
            tc.tile_pool(name="mpool", bufs=1) if mode == "general" else ExitStack()
        ) as mpool:
            if mode == "general":
                mask_sb = mpool.tile([128, JC, T], BF16, name="mask_sb")
                nc.gpsimd.dma_start(
                    mask_sb[:], maskT[:, :, :].rearrange("j p w -> p j w")
                )

            def normalize_start(b, h, half, y_acc):
                """Copy den row out, kick the reciprocal round-trip. Returns
                state for normalize_finish. For half 1 also copy y to SBUF so
                the y_acc slot frees before the next head's first PV."""
                den_sb = npool.tile([1, 1024], F32, name="den_sb", tag="den")
                nc.vector.tensor_copy(den_sb[:], y_acc[DH : DH + 1, :])
                # copy y out of PSUM: frees the y_acc slot early AND the
                # final mul may only read one PSUM operand (rb_ps)
                yu = npool.tile([64, 1024], BF16, name="yu", tag="yu")
                nc.vector.tensor_copy(yu[:], y_acc[0:DH, :])
                # DMA round-trips ride the DVE's own HWDGE queue so they never
                # queue behind AllGather triggers / bulk loads on gpsimd
                den_t = npool.tile([128, 8], F32, name="den_t", tag="den_t")
                nc.scalar.dma_start(den_t[0:128, 0:8], den_sb[0:1, 0:1024])
                r_t = npool.tile([128, 8], F32, name="r_t", tag="r_t")
                nc.vector.reciprocal_approx_fast(r_t[:], den_t[:])
                r_tb = npool.tile([128, 8], BF16, name="r_tb", tag="r_tb")
                nc.vector.tensor_copy(r_tb[:], r_t[:])
                r_row = npool.tile([1, 1024], BF16, name="r_row", tag="r_row")
                nc.scalar.dma_start(r_row[0:1, 0:1024], r_tb[0:128, 0:8])
                return (b, h, half, y_acc, yu, r_row)

            def normalize_finish(state):
                b, h, half, y_acc, yu, r_row = state
                rb_ps = spool.tile([128, 1024], F32, name="rb_ps", tag="S")
                for u in (0, 512):
                    nc.tensor.matmul(
                        rb_ps[0:64, u : u + 512],
                        ones_sb[:],
                        r_row[0:1, u : u + 512],
                        start=True,
                        stop=True,
                    )
                dst = yT_sb[h // 2][
                    64 * (h % 2) : 64 * (h % 2) + 64,
                    b * T + half * 1024 : b * T + half * 1024 + 1024,
                ]
                nc.vector.tensor_mul(dst, yu[:], rb_ps[0:64, :])

            def allgather(b, i):
                nc.gpsimd.dma_start(
                    yT_in[b][i][:], yT_sb[i][:, b * T : (b + 1) * T]
                )
                nc.gpsimd.collective_compute(
                    "AllGather",
                    mybir.AluOpType.bypass,
                    replica_groups=[list(range(NCORES))],
                    ins=[yT_in[b][i].opt()],
                    outs=[yT_all[b][i].opt()],
                )
                # prefetch after AG2 fires: by then AG1 has completed, so the
                # in-order gpsimd queue won't block later den DMAs on it
                if (b, i) == (0, 1):
                    for kc in (0, 2, 4):
                        prefetched[(0, kc)] = load_ysl(0, kc)

            pending = []   # deferred normalize_finish / allgather thunks
            deferred = []
            for b in range(B):
                for h in range(QH):
                    y_acc = [
                        ypsum.tile([DH + 1, 1024], F32, name=f"yacc{c}", tag=f"yacc{c}")
                        for c in range(2)
                    ]
                    for j in range(JC):
                        q0 = qlo(j)
                        pT = ppool.tile([128, T], BF16, name="pT")
                        lk = kT[0:64, b * T + j * 128 : b * T + (j + 1) * 128]
                        for half in range(2):
                            lo, hi = half * 1024, half * 1024 + 1024
                            qs = max(lo, q0)
                            if qs >= hi:
                                continue
                            w = hi - qs
                            S = spool.tile([128, 1024], F32, name="S")
                            for p0 in range(0, w, 512):
                                pw = min(512, w - p0)
                                nc.tensor.matmul(
                                    S[:, p0 : p0 + pw],
                                    lk,
                                    qT[h][
                                        0:64,
                                        b * T + qs + p0 : b * T + qs + p0 + pw,
                                    ],
                                    start=True,
                                    stop=True,
                                )
                            if mode == "general":
                                nc.vector.tensor_add(
                                    S[:, 0:w], S[:, 0:w], mask_sb[:, j, qs:hi]
                                )
                            nc.scalar.activation(pT[:, qs:hi], S[:, 0:w], EXP)
                            if causal and qs == q0:
                                # zero the in-block triangle: keep col qq of
                                # partition kk iff qq >= kk
                                nc.vector.tensor_mul(
                                    pT[:, q0 : q0 + 128],
                                    pT[:, q0 : q0 + 128],
                                    tri01[:],
                                )
                        for half in range(2):
                            lo, hi = half * 1024, half * 1024 + 1024
                            qs = max(lo, q0)
                            if qs >= hi:
                                continue
                            a = qs - lo
                            bounds = [a, 512, 1024] if a < 512 else [a, 1024]
                            pieces = list(zip(bounds, bounds[1:]))
                            if qs == q0:
                                # diag piece last: its pT slice waits on the
                                # triangle zeroing; the other piece doesn't
                                pieces = pieces[::-1]
                            for pa, pb in pieces:
                                nc.tensor.matmul(
                                    y_acc[half][:, pa:pb],
                                    vones[:, b, j, :],
                                    pT[:, lo + pa : lo + pb],
                                    start=(j == 0),
                                    stop=(j == JC - 1 or (half == 0 and j == 7)),
                                    skip_group_check=True,
                                )
                        # half 0 complete once j==7 has accumulated (causal);
                        # start its normalization so the reciprocal round-trip
                        # hides under the remaining key chunks.
                        if causal and j == 7:
                            pending.append(normalize_start(b, h, 0, y_acc[0]))
                        if causal and j == 10 and pending:
                            normalize_finish(pending.pop(0))
                            # the AG deferred from the previous head-pair only
                            # needed that finish — fire it now, not at head end
                            for thunk in deferred:
                                thunk()
                            deferred = []
                    if not causal:
                        pending.append(normalize_start(b, h, 0, y_acc[0]))
                    pending.append(normalize_start(b, h, 1, y_acc[1]))
                    # Flush previous head's deferred finishes now that this
                    # head's scores give the chains time to land.
                    while len(pending) > (1 if (b, h) != (B - 1, QH - 1) else 0):
                        normalize_finish(pending.pop(0))
                    for thunk in deferred:
                        thunk()
                    deferred = []
                    if (b, h) == (B - 1, QH - 1):
                        while pending:
                            normalize_finish(pending.pop(0))
                    if h % 2 == 1:
                        bb, ii = b, h // 2
                        if (b, h) == (B - 1, QH - 1):
                            allgather(bb, ii)
                        else:
                            deferred.append(lambda bb=bb, ii=ii: allgather(bb, ii))
            for thunk in deferred:
                thunk()

        # ------------- phase 4: output projection (out^T shard) --------
        with tc.tile_pool(
            name="popool", bufs=1, space="PSUM"
        ) as pop, tc.tile_pool(name="osb", bufs=2) as osp:
            for b in range(B):
                pos = [
                    pop.tile([128, T], F32, name=f"po{m}", tag=f"po{m}")
                    for m in range(2)
                ]
                # k-chunks in AllGather arrival order: evens (head pair 0 of
                # each core) landed first, odds second.
                korder = [2 * g for g in range(NCORES)] + [
                    2 * g + 1 for g in range(NCORES)
                ]
                for idx, kc in enumerate(korder):
                    ysl = prefetched.pop((b, kc), None)
                    if ysl is None:
                        ysl = load_ysl(b, kc)
                    for m in range(2):
                        for u in range(0, T, 512):
                            nc.tensor.matmul(
                                pos[m][:, u : u + 512],
                                wo_sb[:, kc, m * 128 : (m + 1) * 128],
                                ysl[:, u : u + 512],
                                start=(idx == 0),
                                stop=(idx == KC - 1),
                            )
                for m in range(2):
                    osb = osp.tile([128, T], BF16, name="osb")
                    nc.vector.tensor_copy(osb[:], pos[m][:])
                    nc.gpsimd.dma_start(
                        outT[m, :, b * T : (b + 1) * T], osb[:]
                    )
    if compile:
        nc.compile()
    return nc


_CACHE: dict = {}


def _get_compiled(mode: str) -> bacc.Bacc:
    if mode not in _CACHE:
        _CACHE[mode] = _build(mode)
    return _CACHE[mode]


def _prep_inputs(x, attn_mask, Wq, Wk, Wv, Wo, mode):
    x = np.asarray(x, dtype=np.float32)
    Wq = np.asarray(Wq, dtype=np.float32) * 0.125  # fold 1/sqrt(64) into Wq
    Wk = np.asarray(Wk, dtype=np.float32)
    Wv = np.asarray(Wv, dtype=np.float32)
    Wo = np.asarray(Wo, dtype=np.float32)

    xT = (
        np.ascontiguousarray(x.transpose(0, 2, 1))
        .reshape(B, KC, 128, T)
        .astype(NPBF16)
    )
    maskT = None
    if mode == "general":
        mask2d = np.asarray(attn_mask, dtype=np.float32).reshape(T, T)
        maskT = np.stack(
            [mask2d[:, 128 * j : 128 * (j + 1)].T for j in range(JC)]
        ).astype(NPBF16)

    in_maps = []
    for c in range(NCORES):
        wq_c = np.ascontiguousarray(Wq[:, c * DQ : (c + 1) * DQ]).reshape(
            KC, 128, DQ
        ).astype(NPBF16)
        wkv_c = np.concatenate(
            [Wk[:, c * DH : (c + 1) * DH], Wv[:, c * DH : (c + 1) * DH]], axis=1
        ).reshape(KC, 128, 128).astype(NPBF16)
        wo_c = np.ascontiguousarray(Wo[:, c * DQ : (c + 1) * DQ]).reshape(
            KC, 128, DQ
        ).astype(NPBF16)
        im = {
            "xT": xT,
            "wq": wq_c,
            "wkv": wkv_c,
            "wo": wo_c,
            "ident": np.eye(128, dtype=NPBF16),
        }
        if maskT is not None:
            im["maskT"] = maskT
        in_maps.append(im)
    return in_maps


def _mask_mode(attn_mask) -> str:
    mask2d = np.asarray(attn_mask, dtype=np.float32).reshape(T, T)
    if not mask2d.any():
        return "zeros"
    ref = np.triu(np.full((T, T), -1e9, dtype=np.float32), k=1)
    if np.array_equal(mask2d, ref):
        return "causal"
    return "general"


def _run(x, attn_mask, Wq, Wk, Wv, Wo, trace=False, trace_cores=None):
    mode = _mask_mode(attn_mask)
    nc = _get_compiled(mode)
    in_maps = _prep_inputs(x, attn_mask, Wq, Wk, Wv, Wo, mode)
    res = run_bass_kernel_spmd(
        nc,
        in_maps,
        core_ids=list(range(NCORES)),
        trace=trace,
        trace_cores=trace_cores,
    )
    outT = np.concatenate(
        [
            np.asarray(r["outT"]).astype(np.float32).reshape(DQ, NT)
            for r in res.results
        ],
        axis=0,
    )
    out = np.ascontiguousarray(outT.T).reshape(B, T, HID).astype(np.float32)
    return out, res


def kernel(x, attn_mask, Wq, Wk, Wv, Wo):
    out, _ = _run(x, attn_mask, Wq, Wk, Wv, Wo)
    return out


if __name__ == "__main__":
    rng = np.random.default_rng(0)
    x = rng.standard_normal((B, T, HID), dtype=np.float32)
    mask = np.triu(np.full((T, T), -1e9, dtype=np.float32), k=1)[None, None]
    s = 1.0 / np.sqrt(HID)
    Wq = rng.standard_normal((HID, HEADS * DH), dtype=np.float32) * s
    Wk = rng.standard_normal((HID, KV_HEADS * DH), dtype=np.float32) * s
    Wv = rng.standard_normal((HID, KV_HEADS * DH), dtype=np.float32) * s
    Wo = rng.standard_normal((HEADS * DH, HID), dtype=np.float32) * s
    out = kernel(x, mask, Wq, Wk, Wv, Wo)
    print("out", out.shape, out.dtype, np.abs(out).mean())
